# revision 26
# baseline (speedup 1.0000x reference)
"""Trainium2 Bass kernel for nn_DeepVoxels (octree prune + mean-pool round trip).

Self-contained: takes full octree [1, 64, 299592] f32, shards 8 features per
NeuronCore across 8 cores, returns full [1, 64, 299592] f32.

Closed form of the reference (derived analytically, numerically verified):
  keep-mask cascade over 5 levels (parents at v[dl:2dl], quirky child map):
    k_i[j] = (v[dl_i+j] >= EPS) * k_parent, parent = prev-level tail (j<start)
             or same-level head block j//8 (j>=start)
  out[7607:8192)    = k3means tail = mean-8 of k4means[0:4680)
  out[32768:65536)  = k4means = mean-8 of (leaf * EM)
  out[65536:299592) = (leaf * rep8(keep4))[28088:]
  everything else 0, where EM[m] = keep4[4680+m] for m<28088 (identity:
  keep4[4680+m] = (leaf[m]>=EPS)*keep4[m//8]), EM[m] = rep8(keep4)[m] else.

Synchronization notes (hard-won):
  * raw bass: DVE->DVE RAW needs drain() between ops; DVE->DMA needs
    drain().then_inc(sem) (then_inc on a compute op fires at retire, before
    SBUF writes commit).
  * DMA completion counting on a shared semaphore is only sound when the
    wait covers ALL DMAs issued on that semaphore so far and no later DMA
    on the same semaphore is already in flight (SDMA engines process ring
    slices independently, so partial counts can be satisfied by later
    DMAs' increments). Hence dedicated per-stream / per-buffer-parity sems.
"""
import sys

sys.path.insert(0, "/opt/trn_rl_repo")

import numpy as np

OCT = 299592
F = 64
FEATS = 8
N_CORES = 8
EPS = 1e-5

_cache = {}


def _build(reps: int = 1):
    import concourse.bass as bass
    import concourse.mybir as mybir

    F32 = mybir.dt.float32
    ge, mul = mybir.AluOpType.is_ge, mybir.AluOpType.mult
    add = mybir.AluOpType.add
    X = mybir.AxisListType.X

    nc = bass.Bass()
    x = nc.dram_tensor("x", [FEATS, OCT], F32, kind="ExternalInput")
    y = nc.dram_tensor("y", [FEATS, OCT], F32, kind="ExternalOutput")
    kd = nc.dram_tensor("kd", [FEATS, 32768], F32)  # keep4, linear per feature

    vc = {"n": 0}
    V = {}

    def vsig(v, name, vs):
        vc["n"] += 1
        V[name] = vc["n"]
        return v.drain().then_inc(vs, 1)

    from contextlib import ExitStack

    with ExitStack() as ctx:
        sm = ctx.enter_context(nc.sbuf_tensor([8, 8192], F32))
        k4c = ctx.enter_context(nc.sbuf_tensor([56, 8192], F32))
        hs = ctx.enter_context(nc.sbuf_tensor([56, 1024], F32))
        L0 = ctx.enter_context(nc.sbuf_tensor([128, 2048], F32))
        L1 = ctx.enter_context(nc.sbuf_tensor([128, 2048], F32))
        E0 = ctx.enter_context(nc.sbuf_tensor([128, 2048], F32))
        E1 = ctx.enter_context(nc.sbuf_tensor([128, 2048], F32))
        K0 = ctx.enter_context(nc.sbuf_tensor([128, 256], F32))
        K1 = ctx.enter_context(nc.sbuf_tensor([128, 256], F32))
        M0 = ctx.enter_context(nc.sbuf_tensor([128, 256], F32))
        M1 = ctx.enter_context(nc.sbuf_tensor([128, 256], F32))
        J0 = ctx.enter_context(nc.sbuf_tensor([19, 32], F32))
        J1 = ctx.enter_context(nc.sbuf_tensor([19, 32], F32))
        s_init = ctx.enter_context(nc.semaphore("s_init"))
        s_load0 = ctx.enter_context(nc.semaphore("s_load0"))
        s_load1 = ctx.enter_context(nc.semaphore("s_load1"))
        s_store0 = ctx.enter_context(nc.semaphore("s_store0"))
        s_store1 = ctx.enter_context(nc.semaphore("s_store1"))
        g_hs = ctx.enter_context(nc.semaphore("g_hs"))
        g_kd = ctx.enter_context(nc.semaphore("g_kd"))
        g_k40 = ctx.enter_context(nc.semaphore("g_k40"))
        g_k41 = ctx.enter_context(nc.semaphore("g_k41"))
        g_p0 = ctx.enter_context(nc.semaphore("g_p0"))
        g_p1 = ctx.enter_context(nc.semaphore("g_p1"))
        a_mj0 = ctx.enter_context(nc.semaphore("a_mj0"))
        a_mj1 = ctx.enter_context(nc.semaphore("a_mj1"))
        vs = ctx.enter_context(nc.semaphore("vs"))
        fin = ctx.enter_context(nc.semaphore("fin"))
        block = ctx.enter_context(nc.Block())
        Lb, Eb, Kb, Mb, Jb = [L0, L1], [E0, E1], [K0, K1], [M0, M1], [J0, J1]
        s_load = [s_load0, s_load1]
        s_store = [s_store0, s_store1]
        g_k4 = [g_k40, g_k41]
        g_p = [g_p0, g_p1]
        a_mj = [a_mj0, a_mj1]

        # per-rep semaphore totals (for the timing variant reps>1)
        S_INIT_T, SLOAD_T, SSTORE_T = 48, 64, 128
        G_HS_T, G_KD_T, G_K4_T, G_P_T, A_MJ_T = 48, 32, 64, 128, 192

        def rep8(src2d, n):
            return src2d[:, :, None].to_broadcast([src2d.shape[0], n, 8])

        def blk(ap2d, n):
            return ap2d.rearrange("p (t e) -> p t e", e=8)

        # ---------------- vector program ----------------
        @block.vector
        def _(v):
          for r in range(reps):
            v.wait_ge(s_init, S_INIT_T * (r + 1))
            # ---- keep-mask cascade (strict chain: drain between ops) ----
            v.tensor_scalar(sm[:, 8:16], sm[:, 8:16], EPS, None, ge)
            v.drain()
            v.tensor_tensor(sm[:, 9:16], sm[:, 9:16], sm[:, 8:9].to_broadcast([8, 7]), mul)
            v.drain()
            v.scalar_tensor_tensor(sm[:, 64:72], sm[:, 64:72], EPS,
                                   sm[:, 15:16].to_broadcast([8, 8]), ge, mul)
            v.drain()
            v.scalar_tensor_tensor(blk(sm[:, 72:128], 7), blk(sm[:, 72:128], 7), EPS,
                                   rep8(sm[:, 64:71], 7), ge, mul)
            v.drain()
            v.scalar_tensor_tensor(blk(sm[:, 512:584], 9), blk(sm[:, 512:584], 9), EPS,
                                   rep8(sm[:, 119:128], 9), ge, mul)
            v.drain()
            v.scalar_tensor_tensor(blk(sm[:, 584:1024], 55), blk(sm[:, 584:1024], 55), EPS,
                                   rep8(sm[:, 512:567], 55), ge, mul)
            v.drain()
            v.scalar_tensor_tensor(blk(sm[:, 4096:4680], 73), blk(sm[:, 4096:4680], 73), EPS,
                                   rep8(sm[:, 951:1024], 73), ge, mul)
            v.drain()
            v.scalar_tensor_tensor(blk(sm[:, 4680:8192], 439), blk(sm[:, 4680:8192], 439), EPS,
                                   rep8(sm[:, 4096:4535], 439), ge, mul)
            v.drain()
            v.scalar_tensor_tensor(blk(k4c[0:8, 0:4680], 585), blk(k4c[0:8, 0:4680], 585), EPS,
                                   rep8(sm[:, 7607:8192], 585), ge, mul)
            v.drain()
            v.scalar_tensor_tensor(blk(k4c[0:8, 4680:8192], 439),
                                   blk(k4c[0:8, 4680:8192], 439), EPS,
                                   rep8(k4c[0:8, 0:439], 439), ge, mul)
            vsig(v, f"headA_{r}", vs)
            v.wait_ge(g_hs, G_HS_T * r + 48)
            v.scalar_tensor_tensor(blk(k4c[32:56, :], 1024), blk(k4c[32:56, :], 1024), EPS,
                                   rep8(hs[32:56, 0:1024], 1024), ge, mul)
            vsig(v, f"tailB_{r}", vs)

            # ---- leaf phase, 2-stage pipeline: embcast(f) then body(f-1) ----
            for f in range(FEATS + 1):
                if f < FEATS:
                    b = f % 2
                    v.wait_ge(g_k4[b], G_K4_T * r + 16 * (f // 2 + 1))
                    v.tensor_copy(blk(Eb[b][:, :], 256), rep8(Kb[b][:, :], 256))
                    vsig(v, f"embcast{f}_{r}", vs)
                if f >= 1:
                    g = f - 1
                    b = g % 2
                    v.wait_ge(g_p[b], G_P_T * r + 32 * (g // 2 + 1))
                    v.wait_ge(s_load[b], SLOAD_T * r + 16 * (g // 2 + 1))
                    amt = A_MJ_T * r + (48 * ((g - 2) // 2 + 1) if g >= 2 else 0)
                    if amt > 0:
                        v.wait_ge(a_mj[b], amt)
                    v.tensor_tensor(blk(Lb[b][:, :], 256), blk(Lb[b][:, :], 256),
                                    blk(Eb[b][:, :], 256), mul)
                    v.drain()
                    v.tensor_reduce(Mb[b][:, :], blk(Lb[b][:, :], 256), X, add)
                    vsig(v, f"mask{g}_{r}", vs)
                    v.tensor_scalar(Mb[b][:, :], Mb[b][:, :], 0.125, None, mul)
                    v.drain()
                    v.tensor_reduce(Jb[b][:, :], blk(Mb[b][0:19, :], 32), X, add)
                    v.drain()
                    v.tensor_scalar(Jb[b][:, :], Jb[b][:, :], 0.125, None, mul)
                    vsig(v, f"feat{g}_{r}", vs)
          v.wait_ge(fin, 3)
          for s in (s_init, s_load0, s_load1, s_store0, s_store1, g_hs, g_kd,
                    g_k40, g_k41, g_p0, g_p1, a_mj0, a_mj1, vs, fin):
              v.sem_clear(s)

        # ---------------- sync ring: bulk loads + leaf stores ----------------
        @block.sync
        def _(sync):
          for r in range(reps):
            sync.dma_start(sm[:, :], x[:, 0:8192]).then_inc(s_init, 16)
            sync.dma_start(k4c[0:8, :], x[:, 32768:40960]).then_inc(s_init, 16)
            p4b = x[:, 40960:65536].rearrange("f (c w) -> c f w", w=8192)
            sync.dma_start(k4c[32:56, :], p4b).then_inc(s_init, 16)
            for f in range(2):
                if r > 0:
                    sync.wait_ge(s_store[f], SSTORE_T * r)
                sync.dma_start(Lb[f][:, :], x[f, 37448:299592]).then_inc(s_load[f], 16)
            for f in range(FEATS):
                b = f % 2
                sync.wait_ge(vs, V[f"mask{f}_{r}"])
                sync.dma_start(y[f, 65536:66120], Lb[b][13:14, 1464:2048]).then_inc(s_store[b], 16)
                sync.dma_start(y[f, 66120:299592], Lb[b][14:128, :]).then_inc(s_store[b], 16)
                if f + 2 < FEATS:
                    # stores of f must finish reading Lb[b] before the reload
                    sync.wait_ge(s_store[b], SSTORE_T * r + 32 * (f // 2 + 1))
                    sync.dma_start(Lb[b][:, :], x[f + 2, 37448:299592]).then_inc(s_load[b], 16)
            sync.wait_ge(s_store[0], SSTORE_T * (r + 1))
            sync.wait_ge(s_store[1], SSTORE_T * (r + 1))
          sync.sem_inc(fin, 1)

        # -------- gpsimd ring: SBUF shuffles, kd spill, K4f + EM patches --------
        @block.gpsimd
        def _(gpsimd):
          for r in range(reps):
            gpsimd.wait_ge(vs, V[f"headA_{r}"])
            for c in (1, 2, 3):
                gpsimd.dma_start(
                    hs[32 + 8 * (c - 1) : 32 + 8 * c, 0:1024],
                    k4c[0:8, 439 + 1024 * (c - 1) : 439 + 1024 * c],
                ).then_inc(g_hs, 16)
            gpsimd.wait_ge(vs, V[f"tailB_{r}"])
            gpsimd.dma_start(kd[:, 0:8192], k4c[0:8, :]).then_inc(g_kd, 16)
            kdb = kd[:, 8192:32768].rearrange("f (c w) -> c f w", w=8192)
            gpsimd.dma_start(kdb, k4c[32:56, :]).then_inc(g_kd, 16)
            gpsimd.wait_ge(g_kd, G_KD_T * (r + 1))  # kd fully written before readback
            for f in range(FEATS):
                b = f % 2
                gpsimd.dma_start(Kb[b][:, :], kd[f, :]).then_inc(g_k4[b], 16)
                gpsimd.wait_ge(vs, V[f"embcast{f}_{r}"])
                gpsimd.dma_start(Eb[b][0:13, :], kd[f, 4680:31304]).then_inc(g_p[b], 16)
                gpsimd.dma_start(Eb[b][13:14, 0:1464], kd[f, 31304:32768]).then_inc(g_p[b], 16)
          gpsimd.wait_ge(g_p[0], G_P_T * reps)
          gpsimd.wait_ge(g_p[1], G_P_T * reps)
          gpsimd.wait_ge(g_k4[0], G_K4_T * reps)
          gpsimd.wait_ge(g_k4[1], G_K4_T * reps)
          gpsimd.sem_inc(fin, 1)

        # ------------- act ring: zeros + means/k3means stores -------------
        @block.scalar
        def _(scalar):
          # zero regions of y are covered by the donated zero-initialized
          # output buffers (run_bass_via_pjrt / run_bass_kernel_spmd both
          # pre-zero ExternalOutputs)
          for r in range(reps):
            for f in range(FEATS):
                b = f % 2
                scalar.wait_ge(vs, V[f"feat{f}_{r}"])
                scalar.dma_start(y[f, 32768:65536], Mb[b][:, :]).then_inc(a_mj[b], 16)
                scalar.dma_start(y[f, 7607:8183], Jb[b][0:18, :]).then_inc(a_mj[b], 16)
                scalar.dma_start(y[f, 8183:8192], Jb[b][18:19, 0:9]).then_inc(a_mj[b], 16)
            scalar.wait_ge(a_mj[0], A_MJ_T * (r + 1))
            scalar.wait_ge(a_mj[1], A_MJ_T * (r + 1))
          scalar.sem_inc(fin, 1)

    return nc


def kernel(octree: np.ndarray) -> np.ndarray:
    from concourse.bass_utils import run_bass_kernel_spmd

    octree = np.ascontiguousarray(octree, dtype=np.float32)
    assert octree.shape == (1, F, OCT)

    if "nc" not in _cache:
        _cache["nc"] = _build()
    nc = _cache["nc"]

    in_maps = [
        {"x": octree[0, c * FEATS : (c + 1) * FEATS, :]} for c in range(N_CORES)
    ]
    res = run_bass_kernel_spmd(nc, in_maps, core_ids=list(range(N_CORES)))
    _cache["last"] = res
    out = np.empty((1, F, OCT), np.float32)
    for c in range(N_CORES):
        out[0, c * FEATS : (c + 1) * FEATS, :] = res.results[c]["y"]
    return out


# revision 30
# speedup vs baseline: 5.5307x; 5.5307x over previous
"""Trainium2 Bass kernel for nn_DeepVoxels (octree prune + mean-pool round trip).

Self-contained: takes full octree [1, 64, 299592] f32, shards 8 features per
NeuronCore across 8 cores, returns full [1, 64, 299592] f32.

Closed form of the reference (derived analytically, numerically verified):
  keep-mask cascade over 5 levels (parents at v[dl:2dl], quirky child map):
    k_i[j] = (v[dl_i+j] >= EPS) * k_parent, parent = prev-level tail (j<start)
             or same-level head block j//8 (j>=start)
  out[7607:8192)    = k3means tail = mean-8 of k4means[0:4680)
  out[32768:65536)  = k4means = mean-8 of (leaf * EM)
  out[65536:299592) = (leaf * rep8(keep4))[28088:]
  everything else 0, where EM[m] = keep4[4680+m] for m<28088 (identity:
  keep4[4680+m] = (leaf[m]>=EPS)*keep4[m//8]), EM[m] = rep8(keep4)[m] else.

Synchronization notes (hard-won):
  * raw bass: DVE->DVE RAW needs drain() between ops; DVE->DMA needs
    drain().then_inc(sem) (then_inc on a compute op fires at retire, before
    SBUF writes commit).
  * DMA completion counting on a shared semaphore is only sound when the
    wait covers ALL DMAs issued on that semaphore so far and no later DMA
    on the same semaphore is already in flight (SDMA engines process ring
    slices independently, so partial counts can be satisfied by later
    DMAs' increments). Hence dedicated per-stream / per-buffer-parity sems.
"""
import sys

sys.path.insert(0, "/opt/trn_rl_repo")

import numpy as np

OCT = 299592
F = 64
FEATS = 8
N_CORES = 8
EPS = 1e-5

_cache = {}


def _build(reps: int = 1):
    import concourse.bass as bass
    import concourse.mybir as mybir

    F32 = mybir.dt.float32
    ge, mul = mybir.AluOpType.is_ge, mybir.AluOpType.mult
    add = mybir.AluOpType.add
    X = mybir.AxisListType.X

    nc = bass.Bass()
    x = nc.dram_tensor("x", [FEATS, OCT], F32, kind="ExternalInput")
    y = nc.dram_tensor("y", [FEATS, OCT], F32, kind="ExternalOutput")
    kd = nc.dram_tensor("kd", [FEATS, 32768], F32)  # keep4, linear per feature

    vc = {"n": 0}
    V = {}

    def vsig(v, name, vs):
        vc["n"] += 1
        V[name] = vc["n"]
        return v.drain().then_inc(vs, 1)

    from contextlib import ExitStack

    with ExitStack() as ctx:
        sm = ctx.enter_context(nc.sbuf_tensor([8, 8192], F32))
        k4c = ctx.enter_context(nc.sbuf_tensor([56, 8192], F32))
        hs = ctx.enter_context(nc.sbuf_tensor([56, 1024], F32))
        L0 = ctx.enter_context(nc.sbuf_tensor([128, 2048], F32))
        L1 = ctx.enter_context(nc.sbuf_tensor([128, 2048], F32))
        E0 = ctx.enter_context(nc.sbuf_tensor([128, 2048], F32))
        E1 = ctx.enter_context(nc.sbuf_tensor([128, 2048], F32))
        K0 = ctx.enter_context(nc.sbuf_tensor([128, 256], F32))
        K1 = ctx.enter_context(nc.sbuf_tensor([128, 256], F32))
        M0 = ctx.enter_context(nc.sbuf_tensor([128, 256], F32))
        M1 = ctx.enter_context(nc.sbuf_tensor([128, 256], F32))
        J0 = ctx.enter_context(nc.sbuf_tensor([19, 32], F32))
        J1 = ctx.enter_context(nc.sbuf_tensor([19, 32], F32))
        s_init = ctx.enter_context(nc.semaphore("s_init"))
        s_load0 = ctx.enter_context(nc.semaphore("s_load0"))
        s_load1 = ctx.enter_context(nc.semaphore("s_load1"))
        s_store0 = ctx.enter_context(nc.semaphore("s_store0"))
        s_store1 = ctx.enter_context(nc.semaphore("s_store1"))
        g_hs = ctx.enter_context(nc.semaphore("g_hs"))
        g_kd = ctx.enter_context(nc.semaphore("g_kd"))
        g_k40 = ctx.enter_context(nc.semaphore("g_k40"))
        g_k41 = ctx.enter_context(nc.semaphore("g_k41"))
        g_p0 = ctx.enter_context(nc.semaphore("g_p0"))
        g_p1 = ctx.enter_context(nc.semaphore("g_p1"))
        a_mj0 = ctx.enter_context(nc.semaphore("a_mj0"))
        a_mj1 = ctx.enter_context(nc.semaphore("a_mj1"))
        a_bc = ctx.enter_context(nc.semaphore("a_bc"))
        vs = ctx.enter_context(nc.semaphore("vs"))
        fin = ctx.enter_context(nc.semaphore("fin"))
        block = ctx.enter_context(nc.Block())
        Lb, Eb, Kb, Mb, Jb = [L0, L1], [E0, E1], [K0, K1], [M0, M1], [J0, J1]
        s_load = [s_load0, s_load1]
        s_store = [s_store0, s_store1]
        g_k4 = [g_k40, g_k41]
        g_p = [g_p0, g_p1]
        a_mj = [a_mj0, a_mj1]

        # per-rep semaphore totals (for the timing variant reps>1)
        S_INIT_T, SLOAD_T, SSTORE_T = 48, 64, 128
        G_HS_T, G_KD_T, G_K4_T, G_P_T, A_MJ_T = 48, 32, 64, 128, 192
        A_BC_T = 8

        def rep8(src2d, n):
            return src2d[:, :, None].to_broadcast([src2d.shape[0], n, 8])

        def blk(ap2d, n):
            return ap2d.rearrange("p (t e) -> p t e", e=8)

        # ---------------- vector program ----------------
        @block.vector
        def _(v):
          for r in range(reps):
            v.wait_ge(s_init, S_INIT_T * (r + 1))
            # ---- keep-mask cascade (strict chain: drain between ops) ----
            v.tensor_scalar(sm[:, 8:16], sm[:, 8:16], EPS, None, ge)
            v.drain()
            v.tensor_tensor(sm[:, 9:16], sm[:, 9:16], sm[:, 8:9].to_broadcast([8, 7]), mul)
            v.drain()
            v.scalar_tensor_tensor(sm[:, 64:72], sm[:, 64:72], EPS,
                                   sm[:, 15:16].to_broadcast([8, 8]), ge, mul)
            v.drain()
            v.scalar_tensor_tensor(blk(sm[:, 72:128], 7), blk(sm[:, 72:128], 7), EPS,
                                   rep8(sm[:, 64:71], 7), ge, mul)
            v.drain()
            v.scalar_tensor_tensor(blk(sm[:, 512:584], 9), blk(sm[:, 512:584], 9), EPS,
                                   rep8(sm[:, 119:128], 9), ge, mul)
            v.drain()
            v.scalar_tensor_tensor(blk(sm[:, 584:1024], 55), blk(sm[:, 584:1024], 55), EPS,
                                   rep8(sm[:, 512:567], 55), ge, mul)
            v.drain()
            v.scalar_tensor_tensor(blk(sm[:, 4096:4680], 73), blk(sm[:, 4096:4680], 73), EPS,
                                   rep8(sm[:, 951:1024], 73), ge, mul)
            v.drain()
            v.scalar_tensor_tensor(blk(sm[:, 4680:8192], 439), blk(sm[:, 4680:8192], 439), EPS,
                                   rep8(sm[:, 4096:4535], 439), ge, mul)
            v.drain()
            v.scalar_tensor_tensor(blk(k4c[0:8, 0:4680], 585), blk(k4c[0:8, 0:4680], 585), EPS,
                                   rep8(sm[:, 7607:8192], 585), ge, mul)
            v.drain()
            v.scalar_tensor_tensor(blk(k4c[0:8, 4680:8192], 439),
                                   blk(k4c[0:8, 4680:8192], 439), EPS,
                                   rep8(k4c[0:8, 0:439], 439), ge, mul)
            vsig(v, f"headA_{r}", vs)
            v.wait_ge(g_hs, G_HS_T * r + 48)
            v.scalar_tensor_tensor(blk(k4c[32:56, :], 1024), blk(k4c[32:56, :], 1024), EPS,
                                   rep8(hs[32:56, 0:1024], 1024), ge, mul)
            vsig(v, f"tailB_{r}", vs)

            # ---- leaf phase, 2-stage pipeline: embcast(f) then body(f-1) ----
            for f in range(FEATS + 1):
                if f < FEATS:
                    b = f % 2
                    v.wait_ge(g_k4[b], G_K4_T * r + 16 * (f // 2 + 1))
                    v.tensor_copy(blk(Eb[b][:, :], 256), rep8(Kb[b][:, :], 256))
                    v.drain().then_inc(a_bc, 1)
                if f >= 1:
                    g = f - 1
                    b = g % 2
                    v.wait_ge(g_p[b], G_P_T * r + 32 * (g // 2 + 1))
                    v.wait_ge(s_load[b], SLOAD_T * r + 16 * (g // 2 + 1))
                    amt = A_MJ_T * r + (48 * ((g - 2) // 2 + 1) if g >= 2 else 0)
                    if amt > 0:
                        v.wait_ge(a_mj[b], amt)
                    v.tensor_tensor(blk(Lb[b][:, :], 256), blk(Lb[b][:, :], 256),
                                    blk(Eb[b][:, :], 256), mul)
                    v.drain()
                    v.tensor_reduce(Mb[b][:, :], blk(Lb[b][:, :], 256), X, add)
                    vsig(v, f"mask{g}_{r}", vs)
                    v.tensor_scalar(Mb[b][:, :], Mb[b][:, :], 0.125, None, mul)
                    v.drain()
                    v.tensor_reduce(Jb[b][:, :], blk(Mb[b][0:19, :], 32), X, add)
                    v.drain()
                    v.tensor_scalar(Jb[b][:, :], Jb[b][:, :], 0.125, None, mul)
                    vsig(v, f"feat{g}_{r}", vs)
          v.wait_ge(fin, 3)
          for s in (s_init, s_load0, s_load1, s_store0, s_store1, g_hs, g_kd,
                    g_k40, g_k41, g_p0, g_p1, a_mj0, a_mj1, a_bc, vs, fin):
              v.sem_clear(s)

        # ---------------- sync ring: bulk loads + leaf stores ----------------
        @block.sync
        def _(sync):
          for r in range(reps):
            sync.dma_start(sm[:, :], x[:, 0:8192]).then_inc(s_init, 16)
            sync.dma_start(k4c[0:8, :], x[:, 32768:40960]).then_inc(s_init, 16)
            p4b = x[:, 40960:65536].rearrange("f (c w) -> c f w", w=8192)
            sync.dma_start(k4c[32:56, :], p4b).then_inc(s_init, 16)
            for f in range(2):
                if r > 0:
                    sync.wait_ge(s_store[f], SSTORE_T * r)
                sync.dma_start(Lb[f][:, :], x[f, 37448:299592]).then_inc(s_load[f], 16)
            for f in range(FEATS):
                b = f % 2
                sync.wait_ge(vs, V[f"mask{f}_{r}"])
                sync.dma_start(y[f, 65536:66120], Lb[b][13:14, 1464:2048]).then_inc(s_store[b], 16)
                sync.dma_start(y[f, 66120:299592], Lb[b][14:128, :]).then_inc(s_store[b], 16)
                if f + 2 < FEATS:
                    # stores of f must finish reading Lb[b] before the reload
                    sync.wait_ge(s_store[b], SSTORE_T * r + 32 * (f // 2 + 1))
                    sync.dma_start(Lb[b][:, :], x[f + 2, 37448:299592]).then_inc(s_load[b], 16)
            sync.wait_ge(s_store[0], SSTORE_T * (r + 1))
            sync.wait_ge(s_store[1], SSTORE_T * (r + 1))
          sync.sem_inc(fin, 1)

        # -------- gpsimd ring: SBUF shuffles, kd spill, K4f + EM patches --------
        @block.gpsimd
        def _(gpsimd):
          for r in range(reps):
            gpsimd.wait_ge(vs, V[f"headA_{r}"])
            for c in (1, 2, 3):
                gpsimd.dma_start(
                    hs[32 + 8 * (c - 1) : 32 + 8 * c, 0:1024],
                    k4c[0:8, 439 + 1024 * (c - 1) : 439 + 1024 * c],
                ).then_inc(g_hs, 16)
            gpsimd.wait_ge(vs, V[f"tailB_{r}"])
            gpsimd.dma_start(kd[:, 0:8192], k4c[0:8, :]).then_inc(g_kd, 16)
            kdb = kd[:, 8192:32768].rearrange("f (c w) -> c f w", w=8192)
            gpsimd.dma_start(kdb, k4c[32:56, :]).then_inc(g_kd, 16)
            gpsimd.wait_ge(g_kd, G_KD_T * (r + 1))  # kd fully written before readback
            for f in range(FEATS):
                b = f % 2
                gpsimd.dma_start(Kb[b][:, :], kd[f, :]).then_inc(g_k4[b], 16)
                gpsimd.wait_ge(a_bc, A_BC_T * r + f + 1)
                gpsimd.dma_start(Eb[b][0:13, :], kd[f, 4680:31304]).then_inc(g_p[b], 16)
                gpsimd.dma_start(Eb[b][13:14, 0:1464], kd[f, 31304:32768]).then_inc(g_p[b], 16)
          gpsimd.wait_ge(g_p[0], G_P_T * reps)
          gpsimd.wait_ge(g_p[1], G_P_T * reps)
          gpsimd.wait_ge(g_k4[0], G_K4_T * reps)
          gpsimd.wait_ge(g_k4[1], G_K4_T * reps)
          gpsimd.sem_inc(fin, 1)

        # ------------- act ring: zeros + means/k3means stores -------------
        @block.scalar
        def _(scalar):
          # zero regions of y are covered by the donated zero-initialized
          # output buffers (run_bass_via_pjrt / run_bass_kernel_spmd both
          # pre-zero ExternalOutputs)
          for r in range(reps):
            for f in range(FEATS):
                b = f % 2
                scalar.wait_ge(vs, V[f"feat{f}_{r}"])
                scalar.dma_start(y[f, 32768:65536], Mb[b][:, :]).then_inc(a_mj[b], 16)
                scalar.dma_start(y[f, 7607:8183], Jb[b][0:18, :]).then_inc(a_mj[b], 16)
                scalar.dma_start(y[f, 8183:8192], Jb[b][18:19, 0:9]).then_inc(a_mj[b], 16)
            scalar.wait_ge(a_mj[0], A_MJ_T * (r + 1))
            scalar.wait_ge(a_mj[1], A_MJ_T * (r + 1))
          scalar.sem_inc(fin, 1)

    return nc


def kernel(octree: np.ndarray) -> np.ndarray:
    from concourse.bass_utils import run_bass_kernel_spmd

    octree = np.ascontiguousarray(octree, dtype=np.float32)
    assert octree.shape == (1, F, OCT)

    if "nc" not in _cache:
        _cache["nc"] = _build()
    nc = _cache["nc"]

    in_maps = [
        {"x": octree[0, c * FEATS : (c + 1) * FEATS, :]} for c in range(N_CORES)
    ]
    res = run_bass_kernel_spmd(nc, in_maps, core_ids=list(range(N_CORES)))
    _cache["last"] = res
    out = np.empty((1, F, OCT), np.float32)
    for c in range(N_CORES):
        out[0, c * FEATS : (c + 1) * FEATS, :] = res.results[c]["y"]
    return out
